# revision 27
# baseline (speedup 1.0000x reference)
"""Trainium2 Bass kernel for nn_CapsuleLayer_39075612459355.

Math background (exact algebra, no approximation):
  The reference einsum 'rcdo,bti->btrco' contracts d and i, which appear in
  only ONE operand each, so
      u_hat[b,t,r,c,o] = Wsum[r,c,o] * usum[b,t]
  with Wsum = sum_d W[0], usum = sum_i ui.  u_hat is rank-1 and the whole
  3-iteration dynamic-routing loop collapses to tiny per-batch tensors:
      c_ij  = softmax_c(b_ij)                         [B,R,C]
      S     = sum_r c_ij * Wsum                       [B,C,O]
      n2    = u2[b,t] * sum_o S^2                     [B,T,C]
      scale = n2/(1+n2)/sqrt(n2+eps)                  [B,T,C]
      H     = sum_t u2 * scale                        [B,C]
      G     = sum_o Wsum * S                          [B,R,C]
      b_ij += G * H
      v     = scale * usum * S                        [B,T,C,O]
  Iteration 0 has b_ij = 0, so c_ij = 1/C and S0 = (1/C) sum_r Wsum is
  batch-independent.

Distribution: data-parallel over batch B=16 across 8 cores (2 batches per
core, so (b,t) = 2*64 = 128 = exactly the SBUF partition count), W replicated.

rsqrt is computed as exp(-0.5*ln(x)): ACT's Rsqrt LUT is banned for accuracy
and Sqrt lives in a different ACT table set than Exp (a ~2.7us table swap per
switch); Ln+Exp+Copy+Identity all live in `natural_log_exp_and_others`.
"""

from contextlib import ExitStack

import numpy as np

import concourse.bacc as bacc
import concourse.bass as bass
import concourse.mybir as mybir
import concourse.tile as tile
from concourse.bass_utils import run_bass_kernel_spmd

F32 = mybir.dt.float32
AF = mybir.ActivationFunctionType
ALU = mybir.AluOpType
AX = mybir.AxisListType

B, T, R, C, D, O = 16, 64, 256, 16, 16, 16
NCORES = 8
BPC = B // NCORES  # batches per core = 2
EPS = 1e-9
N_W_PIECES = 2  # W is loaded in 2 pieces of [128, 4096] (2 MB each)


def _make_consts():
    """Host-side constant operands (selection matrices for TensorE etc.)."""
    ones_k1 = np.ones((1, 128), np.float32)  # lhsT for K=1 broadcast matmuls
    one11 = np.ones((1, 1), np.float32)  # rhs for K=1 "transpose" matmuls
    rsum_w = np.full((128, 1), 1.0 / C, np.float32)  # lhsT: S0 = (1/C) sum_r
    # [p=(b,t), m] = delta_{b(p), m}
    bselT128 = np.zeros((128, 2), np.float32)
    for p in range(128):
        bselT128[p, p // T] = 1.0
    # [(b,c), m] = delta_{b,m}
    bsel32 = np.zeros((32, 2), np.float32)
    # [(b,c), (b',t)] = delta_{b,b'}
    bsel128 = np.zeros((32, 128), np.float32)
    # [(b,c), (c',o)] = delta_{c,c'}
    mask = np.zeros((32, 256), np.float32)
    # [(b,c), c'] = delta_{c,c'}
    mask3 = np.zeros((32, 16), np.float32)
    for bb in range(BPC):
        for c in range(C):
            bsel32[bb * C + c, bb] = 1.0
            bsel128[bb * C + c, bb * T : (bb + 1) * T] = 1.0
            mask[bb * C + c, c * O : (c + 1) * O] = 1.0
            mask3[bb * C + c, c] = 1.0
    # [p, kk*16 + c'] = delta_{c(kk*128+p), c'}  with c(x) = x // O
    mask2 = np.zeros((128, 2 * C), np.float32)
    for p in range(128):
        for kk in range(2):
            mask2[p, kk * C + ((kk * 128 + p) // O)] = 1.0
    idn = np.eye(128, dtype=np.float32)
    ones2 = np.ones((2, 128), np.float32)  # lhsT for K=2 broadcast matmul
    # [b', b*16+c] = delta_{b',b}
    bsel2x32 = np.zeros((2, 2 * C), np.float32)
    for bb in range(BPC):
        bsel2x32[bb, bb * C : (bb + 1) * C] = 1.0
    # [b', p] = delta_{b', p//T}
    bselT2x128 = bselT128.T.copy()
    return dict(
        ones_k1=ones_k1,
        one11=one11,
        rsum_w=rsum_w,
        bselT128=bselT128,
        bsel32=bsel32,
        bsel128=bsel128,
        mask=mask,
        mask2=mask2,
        mask3=mask3,
        idn=idn,
        ones2=ones2,
        bsel2x32=bsel2x32,
        bselT2x128=bselT2x128,
    )


CONSTS = _make_consts()


def _pack_consts():
    """Pack all consts into one [128, N] array (one DMA instead of ten)."""
    offs = {}
    col = 0
    for name, arr in CONSTS.items():
        p, w = arr.shape
        offs[name] = (p, col, w)
        col += w
    packed = np.zeros((128, col), np.float32)
    for name, arr in CONSTS.items():
        p, c0, w = offs[name]
        packed[:p, c0 : c0 + w] = arr
    return packed, offs


CPACK, CPACK_OFFS = _pack_consts()


def _bc(ap, dim, n):
    """unsqueeze(dim) then broadcast to size n along it."""
    return ap.unsqueeze(dim).broadcast_to(
        tuple(list(ap.shape[:dim]) + [n] + list(ap.shape[dim:]))
    )


def build_nc(stage: int = 99) -> bass.Bass:
    """stage < 99 truncates the pipeline for debugging: the kernel early-exits
    after that stage and DMAs an intermediate tile to `out`."""
    nc = bacc.Bacc("TRN2", target_bir_lowering=False, debug=False, num_devices=NCORES)

    ui_h = nc.dram_tensor("ui", [128, C], F32, kind="ExternalInput")
    w_h = nc.dram_tensor("W", [R, C * D * O], F32, kind="ExternalInput")
    const_h = nc.dram_tensor("cpack", list(CPACK.shape), F32, kind="ExternalInput")
    out_h = nc.dram_tensor("out", [128, C * O], F32, kind="ExternalOutput")

    with tile.TileContext(nc) as tc, ExitStack() as ctx:
        _build_body(nc, tc, ctx, ui_h, w_h, const_h, out_h, stage)

    # Bacc.compile() runs the full normalization pipeline; in particular
    # generate_event_semaphores, which splits multi-wait instructions (TRN2
    # allows at most 1 sync wait per instruction) via EventSemaphore joiners.
    #
    # The ACT table chooser greedily alternates between `exp_and_others` and
    # `natural_log` (~2.7us reload each time Exp follows Ln or vice versa).
    # All ACT funcs this kernel uses (Ln, Exp, Copy, Identity) live together
    # in `natural_log_exp_and_others`, so present a table list where only
    # that set has members (keeping list length/order = act_func_set_id map).
    import concourse.hw_specs as hw_specs

    real_tables = hw_specs.get_activation_tables(nc.m.arch)
    assert "natural_log_exp_and_others" in real_tables
    only = {
        name: (funcs if name == "natural_log_exp_and_others" else set())
        for name, funcs in real_tables.items()
    }
    orig = bacc.get_activation_tables
    bacc.get_activation_tables = lambda arch: only
    try:
        nc.compile()
    finally:
        bacc.get_activation_tables = orig
    return nc


def _build_body(nc, tc, ctx, ui_h, w_h, const_h, out_h, stage):
    if True:
        sb = ctx.enter_context(tc.tile_pool(name="sb", bufs=2))
        per = ctx.enter_context(tc.tile_pool(name="per", bufs=1))
        wp = ctx.enter_context(tc.tile_pool(name="wp", bufs=N_W_PIECES))
        ps = ctx.enter_context(tc.tile_pool(name="ps", bufs=2, space="PSUM"))
        ps1 = ctx.enter_context(tc.tile_pool(name="ps1", bufs=1, space="PSUM"))

        # ---- consts + ui first (front of the HWDGE ring, tiny) ----------
        cpk = per.tile(list(CPACK.shape), F32, tag="cpk")
        nc.sync.dma_start(out=cpk[:], in_=const_h[:])
        ct = {
            name: cpk[:p, c0 : c0 + w]
            for name, (p, c0, w) in CPACK_OFFS.items()
        }

        uit = per.tile([128, C], F32, tag="uit")
        nc.sync.dma_start(out=uit[:], in_=ui_h[:])

        # ---- W load + d-reduction, pipelined in 8 pieces -----------------
        # Wsum[p=r%128, rr*256 + c*16 + o] = sum_d W[r,c,d,o]
        wsum = per.tile([128, 2 * C * O], F32, tag="wsum")
        piece_cols = C * D * O  # 4096
        for piece in range(N_W_PIECES):
            rr, q = piece, 0
            wt = wp.tile([128, piece_cols], F32, tag="wt")
            nc.sync.dma_start(
                out=wt[:],
                in_=w_h[rr * 128 : (rr + 1) * 128, q * piece_cols : (q + 1) * piece_cols],
            )
            # reduce over d via contiguous halving tree: [c16][d16][o16]
            t1 = sb.tile([128, 2048], F32, tag="wtr1")
            nc.vector.tensor_add(
                t1[:].rearrange("p (c x) -> p c x", c=16),
                wt[:].rearrange("p (c d x) -> p (c d) x", c=16, d=2)[:, 0::2, :],
                wt[:].rearrange("p (c d x) -> p (c d) x", c=16, d=2)[:, 1::2, :],
            )
            t2 = sb.tile([128, 1024], F32, tag="wtr2")
            nc.vector.tensor_add(
                t2[:].rearrange("p (c x) -> p c x", c=16),
                t1[:].rearrange("p (c d x) -> p (c d) x", c=16, d=2)[:, 0::2, :],
                t1[:].rearrange("p (c d x) -> p (c d) x", c=16, d=2)[:, 1::2, :],
            )
            t3 = sb.tile([128, 512], F32, tag="wtr3")
            nc.vector.tensor_add(
                t3[:].rearrange("p (c x) -> p c x", c=16),
                t2[:].rearrange("p (c d x) -> p (c d) x", c=16, d=2)[:, 0::2, :],
                t2[:].rearrange("p (c d x) -> p (c d) x", c=16, d=2)[:, 1::2, :],
            )
            dst = wsum[:, rr * 256 : (rr + 1) * 256]
            nc.vector.tensor_add(
                dst.rearrange("p (c x) -> p c x", c=16),
                t3[:].rearrange("p (c d x) -> p (c d) x", c=16, d=2)[:, 0::2, :],
                t3[:].rearrange("p (c d x) -> p (c d) x", c=16, d=2)[:, 1::2, :],
            )

        # warm the ACT table set (Ln forces natural_log_exp_and_others,
        # which also holds Exp/Copy/Identity) while the W DMA streams.
        warm_in = per.tile([1, 1], F32, tag="warm")
        nc.vector.memset(warm_in[:], 1.0)
        warm_out = per.tile([1, 1], F32, tag="warm2")
        nc.scalar.activation(warm_out[:], warm_in[:], AF.Ln)

        # per-partition eps bias for Ln(n2 + eps)
        eps_t = per.tile([128, 1], F32, tag="eps")
        nc.vector.memset(eps_t[:], EPS)

        # usum, u2, U2sel
        usum = per.tile([128, 1], F32, tag="usum")
        nc.vector.reduce_sum(usum[:], uit[:], axis=AX.X)
        u2 = per.tile([128, 1], F32, tag="u2")
        nc.vector.tensor_mul(u2[:], usum[:], usum[:])
        u2sel = per.tile([128, 2], F32, tag="u2sel")
        nc.vector.tensor_scalar(u2sel[:], ct["bselT128"], u2[:], None, ALU.mult)

        if stage == 2:
            nc.sync.dma_start(out=out_h[:], in_=wsum[:, :256])
            return

        # ---- Wsum_T build (for the G matmuls): 4x PE transpose -----------
        # WsumT[p=(c,o)%128 of chunk kk, kk*256 + rr*128 + r'] = Wsum[r,c,o]
        wsumT = per.tile([128, 2 * 256], F32, tag="wsumT")
        for rr in range(2):
            for kk in range(2):
                tps = ps1.tile([128, 128], F32, tag="t_tp")
                nc.tensor.transpose(
                    tps[:], wsum[:, rr * 256 + kk * 128 : rr * 256 + (kk + 1) * 128],
                    ct["idn"],
                )
                nc.scalar.copy(wsumT[:, kk * 256 + rr * 128 : kk * 256 + (rr + 1) * 128], tps[:])

        if stage == 3:
            nc.sync.dma_start(out=out_h[:], in_=wsumT[:, :256])
            return

        # ---- iteration 0 (b_ij = 0 -> c_ij = 1/C, S0 batch-independent) --
        # S0[(c,o)] = (1/C) sum_r Wsum[r,(c,o)]
        s0p = ps.tile([1, 256], F32, tag="t_T")
        for rr in range(2):
            nc.tensor.matmul(
                s0p[:], lhsT=ct["rsum_w"], rhs=wsum[:, rr * 256 : (rr + 1) * 256],
                start=(rr == 0), stop=(rr == 1),
            )
        s0sb = sb.tile([1, 256], F32, tag="s0sb")
        nc.scalar.copy(s0sb[:], s0p[:])

        # n2S0[c] = sum_o S0^2
        sq0 = sb.tile([1, 256], F32, tag="sq0")
        nc.vector.tensor_mul(sq0[:], s0sb[:], s0sb[:])
        n2s0 = sb.tile([1, C], F32, tag="n2s0")
        nc.vector.reduce_sum(
            n2s0[:], sq0[:].rearrange("p (c o) -> p c o", c=C), axis=AX.X
        )
        # broadcast to all 128 partitions
        n2b0 = ps.tile([128, C], F32, tag="t_mm2")
        nc.tensor.matmul(
            n2b0[:], lhsT=ct["ones_k1"][:1, :128], rhs=n2s0[:], start=True, stop=True
        )
        n2_0 = sb.tile([128, C], F32, tag="n2t")
        nc.vector.tensor_scalar(n2_0[:], n2b0[:], u2[:], None, ALU.mult)

        def squash_scale(n2t):
            """scale = n2/(1+n2) * exp(-0.5*ln(n2+eps)), all [128, C]."""
            ln_t = sb.tile([128, C], F32, tag="ln_t")
            nc.scalar.activation(ln_t[:], n2t[:], AF.Ln, bias=eps_t[:])
            rs_t = sb.tile([128, C], F32, tag="rs_t")
            nc.scalar.activation(rs_t[:], ln_t[:], AF.Exp, scale=-0.5)
            d_t = sb.tile([128, C], F32, tag="d_t")
            nc.vector.tensor_scalar(d_t[:], n2t[:], 1.0, None, ALU.add)
            r_t = sb.tile([128, C], F32, tag="r_t")
            nc.vector.reciprocal(r_t[:], d_t[:])
            m_t = sb.tile([128, C], F32, tag="m_t")
            nc.vector.tensor_mul(m_t[:], n2t[:], rs_t[:])
            sc = sb.tile([128, C], F32, tag="sc")
            nc.vector.tensor_mul(sc[:], m_t[:], r_t[:])
            return sc

        def make_m2h(scale_t):
            """m2h[p, kk, b, c'] = mask2[p,kk,c'] * H[b,c']
            with H[b,c] = sum_t u2*scale (folds the b_ij update's H factor
            into the Smask operand so G-matmuls emit G*H directly)."""
            hp = ps.tile([2, C], F32, tag="t_mm2")
            nc.tensor.matmul(hp[:], lhsT=u2sel[:], rhs=scale_t[:], start=True, stop=True)
            # block-diagonal replication: hrep[b', b*16+c] = H[b',c]*delta_{b'b}
            hrep = sb.tile([2, 2 * C], F32, tag="hrep")
            nc.vector.tensor_mul(
                hrep[:].rearrange("p (b c) -> p b c", b=2),
                _bc(hp[:], 1, BPC),
                ct["bsel2x32"].rearrange("p (b c) -> p b c", b=2),
            )
            hb = ps.tile([128, 2 * C], F32, tag="t_hb")
            nc.tensor.matmul(hb[:], lhsT=ct["ones2"], rhs=hrep[:], start=True, stop=True)
            m2h = sb.tile([128, 64], F32, tag="m2h")
            nc.vector.tensor_mul(
                m2h[:].rearrange("p (k b c) -> p k b c", k=2, b=2),
                _bc(ct["mask2"].rearrange("p (k c) -> p k c", k=2), 2, BPC),
                _bc(hb[:].rearrange("p (b c) -> p b c", b=2), 1, 2),
            )
            return m2h

        scale0 = squash_scale(n2_0)

        if stage == 4:
            nc.sync.dma_start(out=out_h[:, :C], in_=scale0[:])
            return

        m2h0 = make_m2h(scale0)

        # S0^T broadcast to the (c,o)-partition layout, straight from PSUM
        s0T = ps.tile([128, 2], F32, tag="t_mm2")
        for kk in range(2):
            nc.tensor.matmul(
                s0T[:, kk : kk + 1],
                lhsT=s0sb[:1, kk * 128 : (kk + 1) * 128],
                rhs=ct["one11"],
                start=True, stop=True,
            )
        # smask0'[p, kk, b, c'] = S0[(c,o)(p,kk)] * mask2 * H0[b,c']
        smask0 = sb.tile([128, 64], F32, tag="smask0")
        nc.vector.tensor_mul(
            smask0[:].rearrange("p (k b c) -> p k b c", k=2, b=2),
            _bc(_bc(s0T[:], 2, BPC), 3, C),
            m2h0[:].rearrange("p (k b c) -> p k b c", k=2, b=2),
        )
        # G-matmuls now produce b1 = G0*H0 directly: [p, rr*32 + b*16 + c]
        g0 = ps.tile([128, 64], F32, tag="t_hb")
        for rr in range(2):
            for kk in range(2):
                nc.tensor.matmul(
                    g0[:, rr * 32 : (rr + 1) * 32],
                    lhsT=wsumT[:, kk * 256 + rr * 128 : kk * 256 + (rr + 1) * 128],
                    rhs=smask0[:, kk * 32 : (kk + 1) * 32],
                    start=(kk == 0), stop=(kk == 1),
                )
        b_ij = per.tile([128, 64], F32, tag="b_ij")
        nc.vector.tensor_copy(b_ij[:], g0[:])

        if stage == 5:
            nc.sync.dma_start(out=out_h[:, :64], in_=b_ij[:])
            return

        # ---- iterations 1, 2 --------------------------------------------
        for it in (1, 2):
            # softmax over c for each (p, rr, b)
            b4 = b_ij[:].rearrange("p (a c) -> p a c", a=4)
            mx = sb.tile([128, 4], F32, tag="mx")
            nc.vector.reduce_max(mx[:], b4, axis=AX.X)
            ex = sb.tile([128, 64], F32, tag="ex")
            nc.vector.tensor_sub(
                ex[:].rearrange("p (a c) -> p a c", a=4), b4, _bc(mx[:], 2, C)
            )
            e = sb.tile([128, 64], F32, tag="e")
            nc.scalar.activation(e[:], ex[:], AF.Exp)
            sm = sb.tile([128, 4], F32, tag="sm")
            nc.vector.reduce_sum(sm[:], e[:].rearrange("p (a c) -> p a c", a=4), axis=AX.X)
            rc = sb.tile([128, 4], F32, tag="rc")
            nc.vector.reciprocal(rc[:], sm[:])
            cij = sb.tile([128, 64], F32, tag="cij")
            nc.vector.tensor_mul(
                cij[:].rearrange("p (a c) -> p a c", a=4),
                e[:].rearrange("p (a c) -> p a c", a=4),
                _bc(rc[:], 2, C),
            )

            # T[(b,c), (c',o)] = sum_r cij * Wsum ; S is its c'=c diagonal
            tp = ps.tile([32, 256], F32, tag="t_T")
            for rr in range(2):
                nc.tensor.matmul(
                    tp[:],
                    lhsT=cij[:, rr * 32 : (rr + 1) * 32],
                    rhs=wsum[:, rr * 256 : (rr + 1) * 256],
                    start=(rr == 0), stop=(rr == 1),
                )
            tmp = sb.tile([32, 256], F32, tag="tmp")
            nc.vector.tensor_mul(tmp[:], tp[:], ct["mask"])

            # n2S[(b,c)] = sum_{(c',o)} tmp^2   (masked entries are 0)
            sqt = sb.tile([32, 256], F32, tag="sqt")
            nc.vector.tensor_mul(sqt[:], tmp[:], tmp[:])
            n2s = sb.tile([32, 1], F32, tag="n2s")
            nc.vector.reduce_sum(n2s[:], sqt[:], axis=AX.X)
            rt = sb.tile([32, C], F32, tag="rt")
            nc.vector.tensor_scalar(rt[:], ct["mask3"], n2s[:], None, ALU.mult)
            n2tp = ps.tile([128, C], F32, tag="t_mm2")
            nc.tensor.matmul(n2tp[:], lhsT=ct["bsel128"], rhs=rt[:], start=True, stop=True)
            n2t = sb.tile([128, C], F32, tag="n2t")
            nc.vector.tensor_scalar(n2t[:], n2tp[:], u2[:], None, ALU.mult)

            scale_t = squash_scale(n2t)

            if it < 2:
                m2h = make_m2h(scale_t)
                # S^T broadcast: SbT[p=(c,o)%128 of kk, kk*2+b] = S[b,(c,o)]
                sbT = ps.tile([128, 4], F32, tag="t_mm2")
                for kk in range(2):
                    nc.tensor.matmul(
                        sbT[:, kk * 2 : (kk + 1) * 2],
                        lhsT=tmp[:, kk * 128 : (kk + 1) * 128],
                        rhs=ct["bsel32"],
                        start=True, stop=True,
                    )
                # smask'[p,kk,b,c'] = S[b,(c,o)] * mask2 * H[b,c']
                smask = sb.tile([128, 64], F32, tag="smask")
                nc.vector.tensor_mul(
                    smask[:].rearrange("p (k b c) -> p k b c", k=2, b=2),
                    _bc(sbT[:].rearrange("p (k b) -> p k b", k=2), 3, C),
                    m2h[:].rearrange("p (k b c) -> p k b c", k=2, b=2),
                )
                g = ps.tile([128, 64], F32, tag="t_hb")
                for rr in range(2):
                    for kk in range(2):
                        nc.tensor.matmul(
                            g[:, rr * 32 : (rr + 1) * 32],
                            lhsT=wsumT[:, kk * 256 + rr * 128 : kk * 256 + (rr + 1) * 128],
                            rhs=smask[:, kk * 32 : (kk + 1) * 32],
                            start=(kk == 0), stop=(kk == 1),
                        )
                b_ij2 = sb.tile([128, 64], F32, tag="b_ij2")
                nc.vector.tensor_add(b_ij2[:], b_ij[:], g[:])
                b_ij = b_ij2
                if stage == 6:
                    nc.sync.dma_start(out=out_h[:, :64], in_=b_ij[:])
                    return
            else:
                # final output v = scale * usum * S
                s2p = ps.tile([2, 256], F32, tag="t_T")
                nc.tensor.matmul(s2p[:], lhsT=ct["bsel32"][:], rhs=tmp[:], start=True, stop=True)
                s2sb = sb.tile([2, 256], F32, tag="s2sb")
                nc.scalar.copy(s2sb[:], s2p[:])
                # Sbc[p, (c,o)] = S[b(p), (c,o)] via K=2 row-select matmul
                sbc = ps.tile([128, 256], F32, tag="t_T")
                nc.tensor.matmul(
                    sbc[:], lhsT=ct["bselT2x128"], rhs=s2sb[:], start=True, stop=True
                )
                su = sb.tile([128, C], F32, tag="su")
                nc.vector.tensor_scalar(su[:], scale_t[:], usum[:], None, ALU.mult)
                v = sb.tile([128, 256], F32, tag="v")
                nc.vector.tensor_mul(
                    v[:].rearrange("p (c o) -> p c o", c=C),
                    _bc(su[:], 2, O),
                    sbc[:].rearrange("p (c o) -> p c o", c=C),
                )
                nc.sync.dma_start(out=out_h[:], in_=v[:])


_NC_CACHE: bass.Bass | None = None


def _get_nc() -> bass.Bass:
    global _NC_CACHE
    if _NC_CACHE is None:
        _NC_CACHE = build_nc()
    return _NC_CACHE


def make_in_maps(ui: np.ndarray, W: np.ndarray) -> list[dict]:
    ui = np.ascontiguousarray(ui, dtype=np.float32)
    w2d = np.ascontiguousarray(W.reshape(R, C * D * O), dtype=np.float32)
    in_maps = []
    for k in range(NCORES):
        m = {"cpack": CPACK}
        m["ui"] = np.ascontiguousarray(
            ui[k * BPC : (k + 1) * BPC].reshape(BPC * T, C)
        )
        m["W"] = w2d
        in_maps.append(m)
    return in_maps


def kernel(ui: np.ndarray, W: np.ndarray) -> np.ndarray:
    nc = _get_nc()
    res = run_bass_kernel_spmd(nc, make_in_maps(ui, W), list(range(NCORES)))
    outs = [res.results[k]["out"].reshape(BPC, T, C, O) for k in range(NCORES)]
    return np.ascontiguousarray(np.concatenate(outs, axis=0), dtype=np.float32)


# revision 28
# speedup vs baseline: 1.0334x; 1.0334x over previous
"""Trainium2 Bass kernel for nn_CapsuleLayer_39075612459355.

Math background (exact algebra, no approximation):
  The reference einsum 'rcdo,bti->btrco' contracts d and i, which appear in
  only ONE operand each, so
      u_hat[b,t,r,c,o] = Wsum[r,c,o] * usum[b,t]
  with Wsum = sum_d W[0], usum = sum_i ui.  u_hat is rank-1 and the whole
  3-iteration dynamic-routing loop collapses to tiny per-batch tensors:
      c_ij  = softmax_c(b_ij)                         [B,R,C]
      S     = sum_r c_ij * Wsum                       [B,C,O]
      n2    = u2[b,t] * sum_o S^2                     [B,T,C]
      scale = n2/(1+n2)/sqrt(n2+eps)                  [B,T,C]
      H     = sum_t u2 * scale                        [B,C]
      G     = sum_o Wsum * S                          [B,R,C]
      b_ij += G * H
      v     = scale * usum * S                        [B,T,C,O]
  Iteration 0 has b_ij = 0, so c_ij = 1/C and S0 = (1/C) sum_r Wsum is
  batch-independent.

Distribution: data-parallel over batch B=16 across 8 cores (2 batches per
core, so (b,t) = 2*64 = 128 = exactly the SBUF partition count), W replicated.

rsqrt is computed as exp(-0.5*ln(x)): ACT's Rsqrt LUT is banned for accuracy
and Sqrt lives in a different ACT table set than Exp (a ~2.7us table swap per
switch); Ln+Exp+Copy+Identity all live in `natural_log_exp_and_others`.
"""

from contextlib import ExitStack

import numpy as np

import concourse.bacc as bacc
import concourse.bass as bass
import concourse.mybir as mybir
import concourse.tile as tile
from concourse.bass_utils import run_bass_kernel_spmd

F32 = mybir.dt.float32
AF = mybir.ActivationFunctionType
ALU = mybir.AluOpType
AX = mybir.AxisListType

B, T, R, C, D, O = 16, 64, 256, 16, 16, 16
NCORES = 8
BPC = B // NCORES  # batches per core = 2
EPS = 1e-9
N_W_PIECES = 4  # W is loaded in 4 pieces of [128, 2048] (1 MB each)


def _make_consts():
    """Host-side constant operands (selection matrices for TensorE etc.)."""
    ones_k1 = np.ones((1, 128), np.float32)  # lhsT for K=1 broadcast matmuls
    one11 = np.ones((1, 1), np.float32)  # rhs for K=1 "transpose" matmuls
    rsum_w = np.full((128, 1), 1.0 / C, np.float32)  # lhsT: S0 = (1/C) sum_r
    # [p=(b,t), m] = delta_{b(p), m}
    bselT128 = np.zeros((128, 2), np.float32)
    for p in range(128):
        bselT128[p, p // T] = 1.0
    # [(b,c), m] = delta_{b,m}
    bsel32 = np.zeros((32, 2), np.float32)
    # [(b,c), (b',t)] = delta_{b,b'}
    bsel128 = np.zeros((32, 128), np.float32)
    # [(b,c), (c',o)] = delta_{c,c'}
    mask = np.zeros((32, 256), np.float32)
    # [(b,c), c'] = delta_{c,c'}
    mask3 = np.zeros((32, 16), np.float32)
    for bb in range(BPC):
        for c in range(C):
            bsel32[bb * C + c, bb] = 1.0
            bsel128[bb * C + c, bb * T : (bb + 1) * T] = 1.0
            mask[bb * C + c, c * O : (c + 1) * O] = 1.0
            mask3[bb * C + c, c] = 1.0
    # [p, kk*16 + c'] = delta_{c(kk*128+p), c'}  with c(x) = x // O
    mask2 = np.zeros((128, 2 * C), np.float32)
    for p in range(128):
        for kk in range(2):
            mask2[p, kk * C + ((kk * 128 + p) // O)] = 1.0
    idn = np.eye(128, dtype=np.float32)
    ones2 = np.ones((2, 128), np.float32)  # lhsT for K=2 broadcast matmul
    # [b', b*16+c] = delta_{b',b}
    bsel2x32 = np.zeros((2, 2 * C), np.float32)
    for bb in range(BPC):
        bsel2x32[bb, bb * C : (bb + 1) * C] = 1.0
    # [b', p] = delta_{b', p//T}
    bselT2x128 = bselT128.T.copy()
    return dict(
        ones_k1=ones_k1,
        one11=one11,
        rsum_w=rsum_w,
        bselT128=bselT128,
        bsel32=bsel32,
        bsel128=bsel128,
        mask=mask,
        mask2=mask2,
        mask3=mask3,
        idn=idn,
        ones2=ones2,
        bsel2x32=bsel2x32,
        bselT2x128=bselT2x128,
    )


CONSTS = _make_consts()


def _pack_consts():
    """Pack all consts into one [128, N] array (one DMA instead of ten)."""
    offs = {}
    col = 0
    for name, arr in CONSTS.items():
        p, w = arr.shape
        offs[name] = (p, col, w)
        col += w
    packed = np.zeros((128, col), np.float32)
    for name, arr in CONSTS.items():
        p, c0, w = offs[name]
        packed[:p, c0 : c0 + w] = arr
    return packed, offs


CPACK, CPACK_OFFS = _pack_consts()


def _bc(ap, dim, n):
    """unsqueeze(dim) then broadcast to size n along it."""
    return ap.unsqueeze(dim).broadcast_to(
        tuple(list(ap.shape[:dim]) + [n] + list(ap.shape[dim:]))
    )


def build_nc(stage: int = 99) -> bass.Bass:
    """stage < 99 truncates the pipeline for debugging: the kernel early-exits
    after that stage and DMAs an intermediate tile to `out`."""
    nc = bacc.Bacc("TRN2", target_bir_lowering=False, debug=False, num_devices=NCORES)

    ui_h = nc.dram_tensor("ui", [128, C], F32, kind="ExternalInput")
    w_h = nc.dram_tensor("W", [R, C * D * O], F32, kind="ExternalInput")
    const_h = nc.dram_tensor("cpack", list(CPACK.shape), F32, kind="ExternalInput")
    out_h = nc.dram_tensor("out", [128, C * O], F32, kind="ExternalOutput")

    with tile.TileContext(nc) as tc, ExitStack() as ctx:
        _build_body(nc, tc, ctx, ui_h, w_h, const_h, out_h, stage)

    # Bacc.compile() runs the full normalization pipeline; in particular
    # generate_event_semaphores, which splits multi-wait instructions (TRN2
    # allows at most 1 sync wait per instruction) via EventSemaphore joiners.
    #
    # The ACT table chooser greedily alternates between `exp_and_others` and
    # `natural_log` (~2.7us reload each time Exp follows Ln or vice versa).
    # All ACT funcs this kernel uses (Ln, Exp, Copy, Identity) live together
    # in `natural_log_exp_and_others`, so present a table list where only
    # that set has members (keeping list length/order = act_func_set_id map).
    import concourse.hw_specs as hw_specs

    real_tables = hw_specs.get_activation_tables(nc.m.arch)
    assert "natural_log_exp_and_others" in real_tables
    only = {
        name: (funcs if name == "natural_log_exp_and_others" else set())
        for name, funcs in real_tables.items()
    }
    orig = bacc.get_activation_tables
    bacc.get_activation_tables = lambda arch: only
    try:
        nc.compile()
    finally:
        bacc.get_activation_tables = orig
    return nc


def _build_body(nc, tc, ctx, ui_h, w_h, const_h, out_h, stage):
    if True:
        sb = ctx.enter_context(tc.tile_pool(name="sb", bufs=2))
        per = ctx.enter_context(tc.tile_pool(name="per", bufs=1))
        wp = ctx.enter_context(tc.tile_pool(name="wp", bufs=N_W_PIECES))
        ps = ctx.enter_context(tc.tile_pool(name="ps", bufs=2, space="PSUM"))
        ps1 = ctx.enter_context(tc.tile_pool(name="ps1", bufs=1, space="PSUM"))

        # ---- consts + ui first (front of the HWDGE ring, tiny) ----------
        cpk = per.tile(list(CPACK.shape), F32, tag="cpk")
        nc.sync.dma_start(out=cpk[:], in_=const_h[:])
        ct = {
            name: cpk[:p, c0 : c0 + w]
            for name, (p, c0, w) in CPACK_OFFS.items()
        }

        uit = per.tile([128, C], F32, tag="uit")
        nc.sync.dma_start(out=uit[:], in_=ui_h[:])

        # ---- W load + d-reduction, pipelined in 8 pieces -----------------
        # Wsum[p=r%128, rr*256 + c*16 + o] = sum_d W[r,c,d,o]
        wsum = per.tile([128, 2 * C * O], F32, tag="wsum")
        piece_cols = C * D * O // 2  # 2048
        for piece in range(N_W_PIECES):
            rr, q = divmod(piece, 2)
            wt = wp.tile([128, piece_cols], F32, tag="wt")
            nc.sync.dma_start(
                out=wt[:],
                in_=w_h[rr * 128 : (rr + 1) * 128, q * piece_cols : (q + 1) * piece_cols],
            )
            # reduce over d via contiguous halving tree: [c8][d16][o16]
            t1 = sb.tile([128, 1024], F32, tag="wtr1")
            nc.vector.tensor_add(
                t1[:].rearrange("p (c x) -> p c x", c=8),
                wt[:].rearrange("p (c d x) -> p (c d) x", c=8, d=2)[:, 0::2, :],
                wt[:].rearrange("p (c d x) -> p (c d) x", c=8, d=2)[:, 1::2, :],
            )
            t2 = sb.tile([128, 512], F32, tag="wtr2")
            nc.vector.tensor_add(
                t2[:].rearrange("p (c x) -> p c x", c=8),
                t1[:].rearrange("p (c d x) -> p (c d) x", c=8, d=2)[:, 0::2, :],
                t1[:].rearrange("p (c d x) -> p (c d) x", c=8, d=2)[:, 1::2, :],
            )
            t3 = sb.tile([128, 256], F32, tag="wtr3")
            nc.vector.tensor_add(
                t3[:].rearrange("p (c x) -> p c x", c=8),
                t2[:].rearrange("p (c d x) -> p (c d) x", c=8, d=2)[:, 0::2, :],
                t2[:].rearrange("p (c d x) -> p (c d) x", c=8, d=2)[:, 1::2, :],
            )
            dst = wsum[:, rr * 256 + q * 128 : rr * 256 + (q + 1) * 128]
            nc.vector.tensor_add(
                dst.rearrange("p (c x) -> p c x", c=8),
                t3[:].rearrange("p (c d x) -> p (c d) x", c=8, d=2)[:, 0::2, :],
                t3[:].rearrange("p (c d x) -> p (c d) x", c=8, d=2)[:, 1::2, :],
            )

        # warm the ACT table set (Ln forces natural_log_exp_and_others,
        # which also holds Exp/Copy/Identity) while the W DMA streams.
        warm_in = per.tile([1, 1], F32, tag="warm")
        nc.vector.memset(warm_in[:], 1.0)
        warm_out = per.tile([1, 1], F32, tag="warm2")
        nc.scalar.activation(warm_out[:], warm_in[:], AF.Ln)

        # per-partition eps bias for Ln(n2 + eps)
        eps_t = per.tile([128, 1], F32, tag="eps")
        nc.vector.memset(eps_t[:], EPS)

        # usum, u2, U2sel
        usum = per.tile([128, 1], F32, tag="usum")
        nc.vector.reduce_sum(usum[:], uit[:], axis=AX.X)
        u2 = per.tile([128, 1], F32, tag="u2")
        nc.vector.tensor_mul(u2[:], usum[:], usum[:])
        u2sel = per.tile([128, 2], F32, tag="u2sel")
        nc.vector.tensor_scalar(u2sel[:], ct["bselT128"], u2[:], None, ALU.mult)

        if stage == 2:
            nc.sync.dma_start(out=out_h[:], in_=wsum[:, :256])
            return

        # ---- Wsum_T build (for the G matmuls): 4x PE transpose -----------
        # WsumT[p=(c,o)%128 of chunk kk, kk*256 + rr*128 + r'] = Wsum[r,c,o]
        wsumT = per.tile([128, 2 * 256], F32, tag="wsumT")
        for rr in range(2):
            for kk in range(2):
                tps = ps1.tile([128, 128], F32, tag="t_tp")
                nc.tensor.transpose(
                    tps[:], wsum[:, rr * 256 + kk * 128 : rr * 256 + (kk + 1) * 128],
                    ct["idn"],
                )
                nc.scalar.copy(wsumT[:, kk * 256 + rr * 128 : kk * 256 + (rr + 1) * 128], tps[:])

        if stage == 3:
            nc.sync.dma_start(out=out_h[:], in_=wsumT[:, :256])
            return

        # ---- iteration 0 (b_ij = 0 -> c_ij = 1/C, S0 batch-independent) --
        # S0[(c,o)] = (1/C) sum_r Wsum[r,(c,o)]
        s0p = ps.tile([1, 256], F32, tag="t_T")
        for rr in range(2):
            nc.tensor.matmul(
                s0p[:], lhsT=ct["rsum_w"], rhs=wsum[:, rr * 256 : (rr + 1) * 256],
                start=(rr == 0), stop=(rr == 1),
            )
        s0sb = sb.tile([1, 256], F32, tag="s0sb")
        nc.scalar.copy(s0sb[:], s0p[:])

        # n2S0[c] = sum_o S0^2
        sq0 = sb.tile([1, 256], F32, tag="sq0")
        nc.vector.tensor_mul(sq0[:], s0sb[:], s0sb[:])
        n2s0 = sb.tile([1, C], F32, tag="n2s0")
        nc.vector.reduce_sum(
            n2s0[:], sq0[:].rearrange("p (c o) -> p c o", c=C), axis=AX.X
        )
        # broadcast to all 128 partitions
        n2b0 = ps.tile([128, C], F32, tag="t_mm2")
        nc.tensor.matmul(
            n2b0[:], lhsT=ct["ones_k1"][:1, :128], rhs=n2s0[:], start=True, stop=True
        )
        n2_0 = sb.tile([128, C], F32, tag="n2t")
        nc.vector.tensor_scalar(n2_0[:], n2b0[:], u2[:], None, ALU.mult)

        def squash_scale(n2t):
            """scale = n2/(1+n2) * exp(-0.5*ln(n2+eps)), all [128, C]."""
            ln_t = sb.tile([128, C], F32, tag="ln_t")
            nc.scalar.activation(ln_t[:], n2t[:], AF.Ln, bias=eps_t[:])
            rs_t = sb.tile([128, C], F32, tag="rs_t")
            nc.scalar.activation(rs_t[:], ln_t[:], AF.Exp, scale=-0.5)
            d_t = sb.tile([128, C], F32, tag="d_t")
            nc.vector.tensor_scalar(d_t[:], n2t[:], 1.0, None, ALU.add)
            r_t = sb.tile([128, C], F32, tag="r_t")
            nc.vector.reciprocal(r_t[:], d_t[:])
            m_t = sb.tile([128, C], F32, tag="m_t")
            nc.vector.tensor_mul(m_t[:], n2t[:], rs_t[:])
            sc = sb.tile([128, C], F32, tag="sc")
            nc.vector.tensor_mul(sc[:], m_t[:], r_t[:])
            return sc

        def make_m2h(scale_t):
            """m2h[p, kk, b, c'] = mask2[p,kk,c'] * H[b,c']
            with H[b,c] = sum_t u2*scale (folds the b_ij update's H factor
            into the Smask operand so G-matmuls emit G*H directly)."""
            hp = ps.tile([2, C], F32, tag="t_mm2")
            nc.tensor.matmul(hp[:], lhsT=u2sel[:], rhs=scale_t[:], start=True, stop=True)
            # block-diagonal replication: hrep[b', b*16+c] = H[b',c]*delta_{b'b}
            hrep = sb.tile([2, 2 * C], F32, tag="hrep")
            nc.vector.tensor_mul(
                hrep[:].rearrange("p (b c) -> p b c", b=2),
                _bc(hp[:], 1, BPC),
                ct["bsel2x32"].rearrange("p (b c) -> p b c", b=2),
            )
            hb = ps.tile([128, 2 * C], F32, tag="t_hb")
            nc.tensor.matmul(hb[:], lhsT=ct["ones2"], rhs=hrep[:], start=True, stop=True)
            m2h = sb.tile([128, 64], F32, tag="m2h")
            nc.vector.tensor_mul(
                m2h[:].rearrange("p (k b c) -> p k b c", k=2, b=2),
                _bc(ct["mask2"].rearrange("p (k c) -> p k c", k=2), 2, BPC),
                _bc(hb[:].rearrange("p (b c) -> p b c", b=2), 1, 2),
            )
            return m2h

        scale0 = squash_scale(n2_0)

        if stage == 4:
            nc.sync.dma_start(out=out_h[:, :C], in_=scale0[:])
            return

        m2h0 = make_m2h(scale0)

        # S0^T broadcast to the (c,o)-partition layout, straight from PSUM
        s0T = ps.tile([128, 2], F32, tag="t_mm2")
        for kk in range(2):
            nc.tensor.matmul(
                s0T[:, kk : kk + 1],
                lhsT=s0sb[:1, kk * 128 : (kk + 1) * 128],
                rhs=ct["one11"],
                start=True, stop=True,
            )
        # smask0'[p, kk, b, c'] = S0[(c,o)(p,kk)] * mask2 * H0[b,c']
        smask0 = sb.tile([128, 64], F32, tag="smask0")
        nc.vector.tensor_mul(
            smask0[:].rearrange("p (k b c) -> p k b c", k=2, b=2),
            _bc(_bc(s0T[:], 2, BPC), 3, C),
            m2h0[:].rearrange("p (k b c) -> p k b c", k=2, b=2),
        )
        # G-matmuls now produce b1 = G0*H0 directly: [p, rr*32 + b*16 + c]
        g0 = ps.tile([128, 64], F32, tag="t_hb")
        for rr in range(2):
            for kk in range(2):
                nc.tensor.matmul(
                    g0[:, rr * 32 : (rr + 1) * 32],
                    lhsT=wsumT[:, kk * 256 + rr * 128 : kk * 256 + (rr + 1) * 128],
                    rhs=smask0[:, kk * 32 : (kk + 1) * 32],
                    start=(kk == 0), stop=(kk == 1),
                )
        b_ij = per.tile([128, 64], F32, tag="b_ij")
        nc.vector.tensor_copy(b_ij[:], g0[:])

        if stage == 5:
            nc.sync.dma_start(out=out_h[:, :64], in_=b_ij[:])
            return

        # ---- iterations 1, 2 --------------------------------------------
        for it in (1, 2):
            # softmax over c for each (p, rr, b)
            b4 = b_ij[:].rearrange("p (a c) -> p a c", a=4)
            mx = sb.tile([128, 4], F32, tag="mx")
            nc.vector.reduce_max(mx[:], b4, axis=AX.X)
            ex = sb.tile([128, 64], F32, tag="ex")
            nc.vector.tensor_sub(
                ex[:].rearrange("p (a c) -> p a c", a=4), b4, _bc(mx[:], 2, C)
            )
            e = sb.tile([128, 64], F32, tag="e")
            nc.scalar.activation(e[:], ex[:], AF.Exp)
            sm = sb.tile([128, 4], F32, tag="sm")
            nc.vector.reduce_sum(sm[:], e[:].rearrange("p (a c) -> p a c", a=4), axis=AX.X)
            rc = sb.tile([128, 4], F32, tag="rc")
            nc.vector.reciprocal(rc[:], sm[:])
            cij = sb.tile([128, 64], F32, tag="cij")
            nc.vector.tensor_mul(
                cij[:].rearrange("p (a c) -> p a c", a=4),
                e[:].rearrange("p (a c) -> p a c", a=4),
                _bc(rc[:], 2, C),
            )

            # T[(b,c), (c',o)] = sum_r cij * Wsum ; S is its c'=c diagonal
            tp = ps.tile([32, 256], F32, tag="t_T")
            for rr in range(2):
                nc.tensor.matmul(
                    tp[:],
                    lhsT=cij[:, rr * 32 : (rr + 1) * 32],
                    rhs=wsum[:, rr * 256 : (rr + 1) * 256],
                    start=(rr == 0), stop=(rr == 1),
                )
            tmp = sb.tile([32, 256], F32, tag="tmp")
            nc.vector.tensor_mul(tmp[:], tp[:], ct["mask"])

            # n2S[(b,c)] = sum_{(c',o)} tmp^2   (masked entries are 0)
            sqt = sb.tile([32, 256], F32, tag="sqt")
            nc.vector.tensor_mul(sqt[:], tmp[:], tmp[:])
            n2s = sb.tile([32, 1], F32, tag="n2s")
            nc.vector.reduce_sum(n2s[:], sqt[:], axis=AX.X)
            rt = sb.tile([32, C], F32, tag="rt")
            nc.vector.tensor_scalar(rt[:], ct["mask3"], n2s[:], None, ALU.mult)
            n2tp = ps.tile([128, C], F32, tag="t_mm2")
            nc.tensor.matmul(n2tp[:], lhsT=ct["bsel128"], rhs=rt[:], start=True, stop=True)
            n2t = sb.tile([128, C], F32, tag="n2t")
            nc.vector.tensor_scalar(n2t[:], n2tp[:], u2[:], None, ALU.mult)

            scale_t = squash_scale(n2t)

            if it < 2:
                m2h = make_m2h(scale_t)
                # S^T broadcast: SbT[p=(c,o)%128 of kk, kk*2+b] = S[b,(c,o)]
                sbT = ps.tile([128, 4], F32, tag="t_mm2")
                for kk in range(2):
                    nc.tensor.matmul(
                        sbT[:, kk * 2 : (kk + 1) * 2],
                        lhsT=tmp[:, kk * 128 : (kk + 1) * 128],
                        rhs=ct["bsel32"],
                        start=True, stop=True,
                    )
                # smask'[p,kk,b,c'] = S[b,(c,o)] * mask2 * H[b,c']
                smask = sb.tile([128, 64], F32, tag="smask")
                nc.vector.tensor_mul(
                    smask[:].rearrange("p (k b c) -> p k b c", k=2, b=2),
                    _bc(sbT[:].rearrange("p (k b) -> p k b", k=2), 3, C),
                    m2h[:].rearrange("p (k b c) -> p k b c", k=2, b=2),
                )
                g = ps.tile([128, 64], F32, tag="t_hb")
                for rr in range(2):
                    for kk in range(2):
                        nc.tensor.matmul(
                            g[:, rr * 32 : (rr + 1) * 32],
                            lhsT=wsumT[:, kk * 256 + rr * 128 : kk * 256 + (rr + 1) * 128],
                            rhs=smask[:, kk * 32 : (kk + 1) * 32],
                            start=(kk == 0), stop=(kk == 1),
                        )
                b_ij2 = sb.tile([128, 64], F32, tag="b_ij2")
                nc.vector.tensor_add(b_ij2[:], b_ij[:], g[:])
                b_ij = b_ij2
                if stage == 6:
                    nc.sync.dma_start(out=out_h[:, :64], in_=b_ij[:])
                    return
            else:
                # final output v = scale * usum * S
                s2p = ps.tile([2, 256], F32, tag="t_T")
                nc.tensor.matmul(s2p[:], lhsT=ct["bsel32"][:], rhs=tmp[:], start=True, stop=True)
                s2sb = sb.tile([2, 256], F32, tag="s2sb")
                nc.scalar.copy(s2sb[:], s2p[:])
                # Sbc[p, (c,o)] = S[b(p), (c,o)] via K=2 row-select matmul
                sbc = ps.tile([128, 256], F32, tag="t_T")
                nc.tensor.matmul(
                    sbc[:], lhsT=ct["bselT2x128"], rhs=s2sb[:], start=True, stop=True
                )
                su = sb.tile([128, C], F32, tag="su")
                nc.vector.tensor_scalar(su[:], scale_t[:], usum[:], None, ALU.mult)
                v = sb.tile([128, 256], F32, tag="v")
                nc.vector.tensor_mul(
                    v[:].rearrange("p (c o) -> p c o", c=C),
                    _bc(su[:], 2, O),
                    sbc[:].rearrange("p (c o) -> p c o", c=C),
                )
                nc.sync.dma_start(out=out_h[:], in_=v[:])


_NC_CACHE: bass.Bass | None = None


def _get_nc() -> bass.Bass:
    global _NC_CACHE
    if _NC_CACHE is None:
        _NC_CACHE = build_nc()
    return _NC_CACHE


def make_in_maps(ui: np.ndarray, W: np.ndarray) -> list[dict]:
    ui = np.ascontiguousarray(ui, dtype=np.float32)
    w2d = np.ascontiguousarray(W.reshape(R, C * D * O), dtype=np.float32)
    in_maps = []
    for k in range(NCORES):
        m = {"cpack": CPACK}
        m["ui"] = np.ascontiguousarray(
            ui[k * BPC : (k + 1) * BPC].reshape(BPC * T, C)
        )
        m["W"] = w2d
        in_maps.append(m)
    return in_maps


def kernel(ui: np.ndarray, W: np.ndarray) -> np.ndarray:
    nc = _get_nc()
    res = run_bass_kernel_spmd(nc, make_in_maps(ui, W), list(range(NCORES)))
    outs = [res.results[k]["out"].reshape(BPC, T, C, O) for k in range(NCORES)]
    return np.ascontiguousarray(np.concatenate(outs, axis=0), dtype=np.float32)
